# revision 11
# baseline (speedup 1.0000x reference)
"""Trainium2 Bass kernel for nn_AttentionMechanism (tanh-MLP attention).

Math (per batch b):
  q[:, b]   = W_h_w @ h_t[b] + W_h_b + W_b                  (host, tiny)
  U[beta,s,b] = sum_c W_w[beta,c] V[c,s,b]                   (PE)
  T = tanh(U + q)     (q folded in as the ACT per-partition bias)
  E[s,b]    = sum_beta bw[beta] T[beta,s,b]                  (PE, output replicated over partitions)
  w = exp(E)          (no max-subtraction needed: |E| <= ||bw||_1 ~ 8)
  P[c,b]    = sum_s w[s,b] V[c,s,b]                          (DVE mul+reduce, 2x mode)
  SE[b]     = sum_s w[s,b]                                   (DVE reduce)
  C[b,0,c]  = sum_cores P / sum_cores SE                     (host, tiny)

Sharding: position-parallel across 8 cores (hp dim, 8 rows each); softmax
combined on host.  Host pre-lays V out per-core as [c, b, s] bf16 (the
sharding-prep copy), so the device DMA reads contiguous runs at full HBM
bandwidth, every matmul rhs is s-contiguous (full PE rate), and the DVE
P stage runs in 2x mode with contiguous reduces.

On-chip: V lives in 16 resident tiles [c-chunk x b-octet] of [128, 8*512]
bf16; compute pipelines over b-groups of 4 while later octets DMA.  Each
matmul is N=512 = one batch x all 512 positions, PSUM tile [128, 4*512]
per beta-chunk covering a b-group; tanh reads each per-batch bank with
bias=q[beta-chunk, b] (fp32).  E uses a column-replicated beta_w lhsT so
exp directly yields partition-replicated w.
"""

import sys
from contextlib import ExitStack

import numpy as np

if "/opt/trn_rl_repo" not in sys.path:
    sys.path.insert(0, "/opt/trn_rl_repo")

import ml_dtypes

BF16 = ml_dtypes.bfloat16

HP, WP, C_DIM, B = 64, 64, 256, 64
BETA, HIDDEN = 512, 512
NCORES = 8
S_CORE = (HP // NCORES) * WP  # 512 positions per core
B_OCT = 4                     # batches per DMA tile
B_G = 2                       # batches per PSUM group / matmul group

_NC_CACHE = {}


def _build_nc(s_core=S_CORE):
    import concourse.bass as bass
    import concourse.bacc as bacc
    import concourse.tile as tile
    import concourse.mybir as mybir
    from concourse.mybir import dt

    AF = mybir.ActivationFunctionType
    ALU = mybir.AluOpType
    AX = mybir.AxisListType
    f32, bf16 = dt.float32, dt.bfloat16

    n_oct = B // B_OCT            # 8 DMA octets per c-chunk
    n_g = B // B_G                # 16 b-groups

    nc = bacc.Bacc("TRN2", target_bir_lowering=False, debug=False,
                   num_devices=NCORES)

    v_d = nc.dram_tensor("v", [C_DIM, B, s_core], bf16, kind="ExternalInput")
    wt_d = nc.dram_tensor("wt", [128, 2 * BETA], bf16, kind="ExternalInput")
    qs_d = nc.dram_tensor("qs", [128, 4 * B], f32, kind="ExternalInput")
    qt_d = nc.dram_tensor("qt", [B, BETA], bf16, kind="ExternalInput")
    ind_d = nc.dram_tensor("ind", [B, B], bf16, kind="ExternalInput")
    bwr_d = nc.dram_tensor("bwr", [128, BETA], bf16, kind="ExternalInput")
    p_d = nc.dram_tensor("p_out", [2, 128, B], f32, kind="ExternalOutput")
    se_d = nc.dram_tensor("se_out", [1, B], f32, kind="ExternalOutput")

    with tile.TileContext(nc) as tc, ExitStack() as ctx:
        cpool = ctx.enter_context(tc.tile_pool(name="const", bufs=1))
        vpool = ctx.enter_context(tc.tile_pool(name="vp", bufs=1))
        tpool = ctx.enter_context(tc.tile_pool(name="tp", bufs=5))
        wpool = ctx.enter_context(tc.tile_pool(name="wp", bufs=2))
        ppool = ctx.enter_context(tc.tile_pool(name="pp", bufs=2))
        apool = ctx.enter_context(tc.tile_pool(name="ap", bufs=1))
        psum = ctx.enter_context(tc.tile_pool(name="ps", bufs=4, space="PSUM"))

        # ---- constants ----
        wt_sb = cpool.tile([128, 2 * BETA], bf16, tag="wt")
        nc.sync.dma_start(wt_sb, wt_d[:])
        qs_sb = cpool.tile([128, 4 * B], f32, tag="qs")
        nc.sync.dma_start(qs_sb, qs_d[:])
        qt_sb = cpool.tile([B, BETA], bf16, tag="qt")
        nc.sync.dma_start(qt_sb, qt_d[:])
        ind_sb = cpool.tile([B, B], bf16, tag="ind")
        nc.sync.dma_start(ind_sb, ind_d[:])
        bwr_sb = cpool.tile([128, BETA], bf16, tag="bwr")
        nc.sync.dma_start(bwr_sb, bwr_d[:])

        # ---- V tiles: [c-chunk][b-octet] resident, DMA'd in octet order ----
        vv = [[None, None] for _ in range(n_oct)]
        for o in range(n_oct):
            for k in range(2):
                t = vpool.tile([128, B_OCT * s_core], bf16, tag=f"v{k}o{o}",
                               name=f"v{k}o{o}")
                nc.sync.dma_start(
                    t, v_d[k * 128:(k + 1) * 128, o * B_OCT:(o + 1) * B_OCT, :])
                vv[o][k] = t.rearrange("p (b s) -> p b s", s=s_core)

        # ---- output accumulators ----
        p_fin = [apool.tile([128, B], f32, tag=f"pfin{k}", name=f"pfin{k}")
                 for k in range(2)]
        se_fin = apool.tile([128, B], f32, tag="sefin")

        for g in range(n_g):
            b_base = g * B_G
            o = b_base // B_OCT            # octet index
            h = (b_base % B_OCT) // B_G    # half-within-octet
            qmm = (g % 2 == 1)   # alternate groups: q via PE matmul + big tanh
            t_tiles = []
            for m in range(4):
                u = psum.tile([128, B_G * 512], f32, tag="acc", name="u")
                for kp in range(2):
                    for b in range(B_G):
                        nc.tensor.matmul(
                            u[:, b * 512:(b + 1) * 512],
                            wt_sb[:, kp * BETA + m * 128:
                                  kp * BETA + (m + 1) * 128],
                            vv[o][kp][:, h * B_G + b, :],
                            start=(kp == 0), stop=(kp == 1 and not qmm))
                if qmm:
                    for b in range(B_G):
                        nc.tensor.matmul(
                            u[:, b * 512:(b + 1) * 512],
                            qt_sb[:, m * 128:(m + 1) * 128],
                            ind_sb[:, b_base + b:b_base + b + 1]
                            .broadcast_to([B, 1, s_core]),
                            start=False, stop=True)
                t_m = tpool.tile([128, B_G * 512], bf16, tag="t", name="t_m")
                if qmm:
                    nc.scalar.activation(t_m, u, AF.Tanh)
                else:
                    for b in range(B_G):
                        nc.scalar.activation(
                            t_m[:, b * 512:(b + 1) * 512],
                            u[:, b * 512:(b + 1) * 512], AF.Tanh,
                            bias=qs_sb[:, m * B + b_base + b:
                                       m * B + b_base + b + 1])
                t_tiles.append(t_m)

            e_rep = psum.tile([128, B_G * 512], f32, tag="acc", name="e_rep")
            for m in range(4):
                for b in range(B_G):
                    nc.tensor.matmul(
                        e_rep[:, b * 512:(b + 1) * 512],
                        bwr_sb[:, m * 128:(m + 1) * 128],
                        t_tiles[m][:, b * 512:(b + 1) * 512],
                        start=(m == 0), stop=(m == 3))
            w_rep = wpool.tile([128, B_G * 512], bf16, tag="w", name="w_rep")
            nc.scalar.activation(w_rep, e_rep, AF.Exp)

            for k in range(2):
                prod = ppool.tile([128, B_G * s_core], bf16, tag="prod",
                                  name="prod")
                for b in range(B_G):
                    nc.vector.affine_mul_reduce(
                        out=prod[:, b * s_core:(b + 1) * s_core],
                        accum_out=p_fin[k][:, b_base + b:b_base + b + 1],
                        in0=vv[o][k][:, h * B_G + b, :],
                        in1=w_rep[:, b * 512:(b + 1) * 512],
                        scale=1.0, bias=0.0)
            sescr = ppool.tile([128, B_G * 512], bf16, tag="sescr",
                               name="sescr")
            for b in range(B_G):
                nc.vector.tensor_scalar(
                    sescr[:, b * 512:(b + 1) * 512],
                    w_rep[:, b * 512:(b + 1) * 512], 1.0, None,
                    op0=ALU.mult, op1=ALU.add,
                    accum_out=se_fin[:, b_base + b:b_base + b + 1])

        for k in range(2):
            nc.sync.dma_start(p_d[k], p_fin[k])
        nc.sync.dma_start(se_d[:], se_fin[0:1, :])

    nc.compile()
    return nc


def _get_nc(s_core=S_CORE):
    if s_core not in _NC_CACHE:
        _NC_CACHE[s_core] = _build_nc(s_core)
    return _NC_CACHE[s_core]


def _host_smalls(h_t, W_h_w, W_h_b, W_w, W_b, beta_w):
    q = h_t[:, 0, :].astype(np.float64) @ W_h_w.T.astype(np.float64) \
        + W_h_b + W_b                                  # [b, beta]
    # qs[p, m*64+b] = q[b, m*128+p]
    qs = np.ascontiguousarray(
        q.T.reshape(4, 128, B).transpose(1, 0, 2).reshape(128, 4 * B)
    ).astype(np.float32)
    wt = np.ascontiguousarray(
        W_w.T.reshape(2, 128, BETA).transpose(1, 0, 2).reshape(128, 2 * BETA)
    ).astype(BF16)
    bw = beta_w[0].astype(np.float32)
    bwr = np.ascontiguousarray(
        np.repeat(bw.reshape(4, 128).T[:, :, None], 128, axis=2).reshape(128, BETA)
    ).astype(BF16)
    qt = np.ascontiguousarray(q).astype(BF16)
    ind = np.eye(B, dtype=np.float32).astype(BF16)
    return qs, wt, bwr, qt, ind


_PROFILE = False
_LAST_PERF = {}


def kernel(**inputs):
    from concourse.bass_utils import run_bass_kernel_spmd

    V = np.asarray(inputs["V"], dtype=np.float32)
    h_t = np.asarray(inputs["h_t"], dtype=np.float32)
    W_h_w = np.asarray(inputs["W_h_w"], dtype=np.float32)
    W_h_b = np.asarray(inputs["W_h_b"], dtype=np.float32)
    W_w = np.asarray(inputs["W_w"], dtype=np.float32)
    W_b = np.asarray(inputs["W_b"], dtype=np.float32)
    beta_w = np.asarray(inputs["beta_w"], dtype=np.float32)
    beta_b = np.asarray(inputs["beta_b"], dtype=np.float32)

    qs, wt, bwr, qt, ind = _host_smalls(h_t, W_h_w, W_h_b, W_w, W_b, beta_w)

    rows = HP // NCORES
    Vb = V.astype(BF16)
    in_maps = []
    for k in range(NCORES):
        # [s, c, b] -> [c, b, s] contiguous (per-core shard layout)
        vk = np.ascontiguousarray(
            Vb[k * rows:(k + 1) * rows].reshape(S_CORE, C_DIM, B)
            .transpose(1, 2, 0))
        in_maps.append({"v": vk, "wt": wt, "qs": qs, "bwr": bwr,
                        "qt": qt, "ind": ind})

    nc = _get_nc()
    res = run_bass_kernel_spmd(nc, in_maps, core_ids=list(range(NCORES)),
                               trace=_PROFILE)
    if _PROFILE:
        _LAST_PERF["exec_time_ns"] = res.exec_time_ns
        _LAST_PERF["trace"] = res.instructions_and_trace
    P = np.zeros((2, 128, B), np.float64)
    SE = np.zeros((B,), np.float64)
    for r in res.results:
        P += r["p_out"]
        SE += r["se_out"][0]
    P = P.reshape(C_DIM, B)
    # softmax is shift-invariant so beta_b cancels; no max-sub needed (|E|<=~8)
    C = (P / SE).T.reshape(B, 1, C_DIM)
    return C.astype(np.float32)


# revision 12
# speedup vs baseline: 1.0620x; 1.0620x over previous
"""Trainium2 Bass kernel for nn_AttentionMechanism (tanh-MLP attention).

Math (per batch b):
  q[:, b]   = W_h_w @ h_t[b] + W_h_b + W_b                  (host, tiny)
  U[beta,s,b] = sum_c W_w[beta,c] V[c,s,b]                   (PE)
  T = tanh(U + q)     (q folded in as the ACT per-partition bias)
  E[s,b]    = sum_beta bw[beta] T[beta,s,b]                  (PE, output replicated over partitions)
  w = exp(E)          (no max-subtraction needed: |E| <= ||bw||_1 ~ 8)
  P[c,b]    = sum_s w[s,b] V[c,s,b]                          (DVE mul+reduce, 2x mode)
  SE[b]     = sum_s w[s,b]                                   (DVE reduce)
  C[b,0,c]  = sum_cores P / sum_cores SE                     (host, tiny)

Sharding: position-parallel across 8 cores (hp dim, 8 rows each); softmax
combined on host.  Host pre-lays V out per-core as [c, b, s] bf16 (the
sharding-prep copy), so the device DMA reads contiguous runs at full HBM
bandwidth, every matmul rhs is s-contiguous (full PE rate), and the DVE
P stage runs in 2x mode with contiguous reduces.

On-chip: V lives in 16 resident tiles [c-chunk x b-octet] of [128, 8*512]
bf16; compute pipelines over b-groups of 4 while later octets DMA.  Each
matmul is N=512 = one batch x all 512 positions, PSUM tile [128, 4*512]
per beta-chunk covering a b-group; tanh reads each per-batch bank with
bias=q[beta-chunk, b] (fp32).  E uses a column-replicated beta_w lhsT so
exp directly yields partition-replicated w.
"""

import sys
from contextlib import ExitStack

import numpy as np

if "/opt/trn_rl_repo" not in sys.path:
    sys.path.insert(0, "/opt/trn_rl_repo")

import ml_dtypes

BF16 = ml_dtypes.bfloat16

HP, WP, C_DIM, B = 64, 64, 256, 64
BETA, HIDDEN = 512, 512
NCORES = 8
S_CORE = (HP // NCORES) * WP  # 512 positions per core
B_OCT = 4                     # batches per DMA tile
B_G = 2                       # batches per PSUM group / matmul group

_NC_CACHE = {}


def _build_nc(s_core=S_CORE):
    import concourse.bass as bass
    import concourse.bacc as bacc
    import concourse.tile as tile
    import concourse.mybir as mybir
    from concourse.mybir import dt

    AF = mybir.ActivationFunctionType
    ALU = mybir.AluOpType
    AX = mybir.AxisListType
    f32, bf16 = dt.float32, dt.bfloat16

    n_oct = B // B_OCT            # 8 DMA octets per c-chunk
    n_g = B // B_G                # 16 b-groups

    nc = bacc.Bacc("TRN2", target_bir_lowering=False, debug=False,
                   num_devices=NCORES)

    v_d = nc.dram_tensor("v", [C_DIM, B, s_core], bf16, kind="ExternalInput")
    wt_d = nc.dram_tensor("wt", [128, 2 * BETA], bf16, kind="ExternalInput")
    qs_d = nc.dram_tensor("qs", [128, 4 * B], f32, kind="ExternalInput")
    qt_d = nc.dram_tensor("qt", [B, BETA], bf16, kind="ExternalInput")
    ind_d = nc.dram_tensor("ind", [B, B], bf16, kind="ExternalInput")
    bwr_d = nc.dram_tensor("bwr", [128, BETA], bf16, kind="ExternalInput")
    p_d = nc.dram_tensor("p_out", [2, 128, B], f32, kind="ExternalOutput")
    se_d = nc.dram_tensor("se_out", [1, B], f32, kind="ExternalOutput")

    with tile.TileContext(nc) as tc, ExitStack() as ctx:
        cpool = ctx.enter_context(tc.tile_pool(name="const", bufs=1))
        vpool = ctx.enter_context(tc.tile_pool(name="vp", bufs=1))
        tpool = ctx.enter_context(tc.tile_pool(name="tp", bufs=5))
        wpool = ctx.enter_context(tc.tile_pool(name="wp", bufs=2))
        ppool = ctx.enter_context(tc.tile_pool(name="pp", bufs=2))
        apool = ctx.enter_context(tc.tile_pool(name="ap", bufs=1))
        psum = ctx.enter_context(tc.tile_pool(name="ps", bufs=4, space="PSUM"))

        # ---- constants ----
        wt_sb = cpool.tile([128, 2 * BETA], bf16, tag="wt")
        nc.sync.dma_start(wt_sb, wt_d[:])
        qs_sb = cpool.tile([128, 4 * B], f32, tag="qs")
        nc.sync.dma_start(qs_sb, qs_d[:])
        qt_sb = cpool.tile([B, BETA], bf16, tag="qt")
        nc.sync.dma_start(qt_sb, qt_d[:])
        ind_sb = cpool.tile([B, B], bf16, tag="ind")
        nc.sync.dma_start(ind_sb, ind_d[:])
        bwr_sb = cpool.tile([128, BETA], bf16, tag="bwr")
        nc.sync.dma_start(bwr_sb, bwr_d[:])

        # ---- V tiles: [c-chunk][b-octet] resident, DMA'd in octet order ----
        vv = [[None, None] for _ in range(n_oct)]
        for o in range(n_oct):
            for k in range(2):
                t = vpool.tile([128, B_OCT * s_core], bf16, tag=f"v{k}o{o}",
                               name=f"v{k}o{o}")
                nc.sync.dma_start(
                    t, v_d[k * 128:(k + 1) * 128, o * B_OCT:(o + 1) * B_OCT, :])
                vv[o][k] = t.rearrange("p (b s) -> p b s", s=s_core)

        # ---- output accumulators ----
        p_fin = [apool.tile([128, B], f32, tag=f"pfin{k}", name=f"pfin{k}")
                 for k in range(2)]
        se_fin = apool.tile([128, B], f32, tag="sefin")

        for g in range(n_g):
            b_base = g * B_G
            o = b_base // B_OCT            # octet index
            h = (b_base % B_OCT) // B_G    # half-within-octet
            qmm = False   # q via ACT bias only (PE mix measured net-negative)
            t_tiles = []
            for m in range(4):
                u = psum.tile([128, B_G * 512], f32, tag="acc", name="u")
                for kp in range(2):
                    for b in range(B_G):
                        nc.tensor.matmul(
                            u[:, b * 512:(b + 1) * 512],
                            wt_sb[:, kp * BETA + m * 128:
                                  kp * BETA + (m + 1) * 128],
                            vv[o][kp][:, h * B_G + b, :],
                            start=(kp == 0), stop=(kp == 1 and not qmm))
                if qmm:
                    for b in range(B_G):
                        nc.tensor.matmul(
                            u[:, b * 512:(b + 1) * 512],
                            qt_sb[:, m * 128:(m + 1) * 128],
                            ind_sb[:, b_base + b:b_base + b + 1]
                            .broadcast_to([B, 1, s_core]),
                            start=False, stop=True)
                t_m = tpool.tile([128, B_G * 512], bf16, tag="t", name="t_m")
                if qmm:
                    nc.scalar.activation(t_m, u, AF.Tanh)
                else:
                    for b in range(B_G):
                        nc.scalar.activation(
                            t_m[:, b * 512:(b + 1) * 512],
                            u[:, b * 512:(b + 1) * 512], AF.Tanh,
                            bias=qs_sb[:, m * B + b_base + b:
                                       m * B + b_base + b + 1])
                t_tiles.append(t_m)

            e_rep = psum.tile([128, B_G * 512], f32, tag="acc", name="e_rep")
            for m in range(4):
                for b in range(B_G):
                    nc.tensor.matmul(
                        e_rep[:, b * 512:(b + 1) * 512],
                        bwr_sb[:, m * 128:(m + 1) * 128],
                        t_tiles[m][:, b * 512:(b + 1) * 512],
                        start=(m == 0), stop=(m == 3))
            w_rep = wpool.tile([128, B_G * 512], bf16, tag="w", name="w_rep")
            nc.scalar.activation(w_rep, e_rep, AF.Exp)

            for k in range(2):
                prod = ppool.tile([128, B_G * s_core], bf16, tag="prod",
                                  name="prod")
                for b in range(B_G):
                    nc.vector.affine_mul_reduce(
                        out=prod[:, b * s_core:(b + 1) * s_core],
                        accum_out=p_fin[k][:, b_base + b:b_base + b + 1],
                        in0=vv[o][k][:, h * B_G + b, :],
                        in1=w_rep[:, b * 512:(b + 1) * 512],
                        scale=1.0, bias=0.0)
            sescr = ppool.tile([128, B_G * 512], bf16, tag="sescr",
                               name="sescr")
            for b in range(B_G):
                nc.vector.tensor_scalar(
                    sescr[:, b * 512:(b + 1) * 512],
                    w_rep[:, b * 512:(b + 1) * 512], 1.0, None,
                    op0=ALU.mult, op1=ALU.add,
                    accum_out=se_fin[:, b_base + b:b_base + b + 1])

        for k in range(2):
            nc.sync.dma_start(p_d[k], p_fin[k])
        nc.sync.dma_start(se_d[:], se_fin[0:1, :])

    nc.compile()
    return nc


def _get_nc(s_core=S_CORE):
    if s_core not in _NC_CACHE:
        _NC_CACHE[s_core] = _build_nc(s_core)
    return _NC_CACHE[s_core]


def _host_smalls(h_t, W_h_w, W_h_b, W_w, W_b, beta_w):
    q = h_t[:, 0, :].astype(np.float64) @ W_h_w.T.astype(np.float64) \
        + W_h_b + W_b                                  # [b, beta]
    # qs[p, m*64+b] = q[b, m*128+p]
    qs = np.ascontiguousarray(
        q.T.reshape(4, 128, B).transpose(1, 0, 2).reshape(128, 4 * B)
    ).astype(np.float32)
    wt = np.ascontiguousarray(
        W_w.T.reshape(2, 128, BETA).transpose(1, 0, 2).reshape(128, 2 * BETA)
    ).astype(BF16)
    bw = beta_w[0].astype(np.float32)
    bwr = np.ascontiguousarray(
        np.repeat(bw.reshape(4, 128).T[:, :, None], 128, axis=2).reshape(128, BETA)
    ).astype(BF16)
    qt = np.ascontiguousarray(q).astype(BF16)
    ind = np.eye(B, dtype=np.float32).astype(BF16)
    return qs, wt, bwr, qt, ind


_PROFILE = False
_LAST_PERF = {}


def kernel(**inputs):
    from concourse.bass_utils import run_bass_kernel_spmd

    V = np.asarray(inputs["V"], dtype=np.float32)
    h_t = np.asarray(inputs["h_t"], dtype=np.float32)
    W_h_w = np.asarray(inputs["W_h_w"], dtype=np.float32)
    W_h_b = np.asarray(inputs["W_h_b"], dtype=np.float32)
    W_w = np.asarray(inputs["W_w"], dtype=np.float32)
    W_b = np.asarray(inputs["W_b"], dtype=np.float32)
    beta_w = np.asarray(inputs["beta_w"], dtype=np.float32)
    beta_b = np.asarray(inputs["beta_b"], dtype=np.float32)

    qs, wt, bwr, qt, ind = _host_smalls(h_t, W_h_w, W_h_b, W_w, W_b, beta_w)

    rows = HP // NCORES
    Vb = V.astype(BF16)
    in_maps = []
    for k in range(NCORES):
        # [s, c, b] -> [c, b, s] contiguous (per-core shard layout)
        vk = np.ascontiguousarray(
            Vb[k * rows:(k + 1) * rows].reshape(S_CORE, C_DIM, B)
            .transpose(1, 2, 0))
        in_maps.append({"v": vk, "wt": wt, "qs": qs, "bwr": bwr,
                        "qt": qt, "ind": ind})

    nc = _get_nc()
    res = run_bass_kernel_spmd(nc, in_maps, core_ids=list(range(NCORES)),
                               trace=_PROFILE)
    if _PROFILE:
        _LAST_PERF["exec_time_ns"] = res.exec_time_ns
        _LAST_PERF["trace"] = res.instructions_and_trace
    P = np.zeros((2, 128, B), np.float64)
    SE = np.zeros((B,), np.float64)
    for r in res.results:
        P += r["p_out"]
        SE += r["se_out"][0]
    P = P.reshape(C_DIM, B)
    # softmax is shift-invariant so beta_b cancels; no max-sub needed (|E|<=~8)
    C = (P / SE).T.reshape(B, 1, C_DIM)
    return C.astype(np.float32)


# revision 13
# speedup vs baseline: 1.2076x; 1.1371x over previous
"""Trainium2 Bass kernel for nn_AttentionMechanism (tanh-MLP attention).

Math (per batch b):
  q[:, b]   = W_h_w @ h_t[b] + W_h_b + W_b                  (host, tiny)
  U[beta,s,b] = sum_c W_w[beta,c] V[c,s,b]                   (PE)
  T = tanh(U + q)     (q folded in as the ACT per-partition bias)
  E[s,b]    = sum_beta bw[beta] T[beta,s,b]                  (PE, output replicated over partitions)
  w = exp(E)          (no max-subtraction needed: |E| <= ||bw||_1 ~ 8)
  P[c,b]    = sum_s w[s,b] V[c,s,b]                          (DVE affine_mul_reduce)
  SE[b]     = sum_s w[s,b]                                   (DVE tensor_scalar accum)
  C[b,0,c]  = sum_cores P / sum_cores SE                     (host, tiny)

Sharding: 2D - 4-way over positions (hp quarters) x 2-way over batch
halves.  Each core gets s=1024 positions x 32 batches (32MB of V);
softmax combined on host over the 4 position-shards of each batch half.
The s=1024 per (core, batch) makes every ACT instruction FD>=1024,
amortizing the per-instruction overhead that bounded the 1D version.

Host pre-lays V out per-core as [c, b, s] bf16 (the sharding-prep copy),
so the device DMA reads contiguous runs at full HBM bandwidth, every
matmul rhs is s-contiguous (full PE rate), and the DVE P stage is a
single fused multiply-accumulate per (c-chunk, batch).
"""

import sys
from contextlib import ExitStack

import numpy as np

if "/opt/trn_rl_repo" not in sys.path:
    sys.path.insert(0, "/opt/trn_rl_repo")

import ml_dtypes

BF16 = ml_dtypes.bfloat16

HP, WP, C_DIM, B = 64, 64, 256, 64
BETA, HIDDEN = 512, 512
NCORES = 8
N_HPQ = 4                      # position shards
N_BH = 2                       # batch shards
B_CORE = B // N_BH             # 32 batches per core
S_CORE = (HP // N_HPQ) * WP    # 1024 positions per core
B_OCT = 2                      # batches per DMA tile

_NC_CACHE = {}


def _build_nc(s_core=S_CORE):
    import concourse.bass as bass
    import concourse.bacc as bacc
    import concourse.tile as tile
    import concourse.mybir as mybir
    from concourse.mybir import dt

    AF = mybir.ActivationFunctionType
    ALU = mybir.AluOpType
    f32, bf16 = dt.float32, dt.bfloat16

    n_oct = B_CORE // B_OCT
    n_sh = s_core // 512           # matmul N=512 tiles per batch

    nc = bacc.Bacc("TRN2", target_bir_lowering=False, debug=False,
                   num_devices=NCORES)

    v_d = nc.dram_tensor("v", [C_DIM, B_CORE, s_core], bf16,
                         kind="ExternalInput")
    wt_d = nc.dram_tensor("wt", [128, 2 * BETA], bf16, kind="ExternalInput")
    qs_d = nc.dram_tensor("qs", [128, 4 * B_CORE], f32, kind="ExternalInput")
    bwr_d = nc.dram_tensor("bwr", [128, BETA], bf16, kind="ExternalInput")
    p_d = nc.dram_tensor("p_out", [2, 128, B_CORE], f32, kind="ExternalOutput")
    se_d = nc.dram_tensor("se_out", [1, B_CORE], f32, kind="ExternalOutput")

    with tile.TileContext(nc) as tc, ExitStack() as ctx:
        cpool = ctx.enter_context(tc.tile_pool(name="const", bufs=1))
        vpool = ctx.enter_context(tc.tile_pool(name="vp", bufs=1))
        tpool = ctx.enter_context(tc.tile_pool(name="tp", bufs=5))
        wpool = ctx.enter_context(tc.tile_pool(name="wp", bufs=2))
        ppool = ctx.enter_context(tc.tile_pool(name="pp", bufs=2))
        apool = ctx.enter_context(tc.tile_pool(name="ap", bufs=1))
        psum = ctx.enter_context(tc.tile_pool(name="ps", bufs=4, space="PSUM"))

        # ---- constants ----
        wt_sb = cpool.tile([128, 2 * BETA], bf16, tag="wt")
        nc.sync.dma_start(wt_sb, wt_d[:])
        qs_sb = cpool.tile([128, 4 * B_CORE], f32, tag="qs")
        nc.sync.dma_start(qs_sb, qs_d[:])
        bwr_sb = cpool.tile([128, BETA], bf16, tag="bwr")
        nc.sync.dma_start(bwr_sb, bwr_d[:])

        # ---- V tiles: [c-chunk][b-pair] resident, DMA'd in order ----
        vv = [[None, None] for _ in range(n_oct)]
        for o in range(n_oct):
            for k in range(2):
                t = vpool.tile([128, B_OCT * s_core], bf16, tag=f"v{k}o{o}",
                               name=f"v{k}o{o}")
                nc.sync.dma_start(
                    t, v_d[k * 128:(k + 1) * 128, o * B_OCT:(o + 1) * B_OCT, :])
                vv[o][k] = t.rearrange("p (b s) -> p b s", s=s_core)

        # ---- output accumulators ----
        p_fin = [apool.tile([128, B_CORE], f32, tag=f"pfin{k}",
                            name=f"pfin{k}") for k in range(2)]
        se_fin = apool.tile([128, B_CORE], f32, tag="sefin")

        for b in range(B_CORE):
            o, h = divmod(b, B_OCT)
            t_tiles = []
            for m in range(4):
                u = psum.tile([128, s_core], f32, tag="acc", name="u")
                for kp in range(2):
                    for sh in range(n_sh):
                        nc.tensor.matmul(
                            u[:, sh * 512:(sh + 1) * 512],
                            wt_sb[:, kp * BETA + m * 128:
                                  kp * BETA + (m + 1) * 128],
                            vv[o][kp][:, h, sh * 512:(sh + 1) * 512],
                            start=(kp == 0), stop=(kp == 1))
                t_m = tpool.tile([128, s_core], bf16, tag="t", name="t_m")
                nc.scalar.activation(
                    t_m, u, AF.Tanh,
                    bias=qs_sb[:, m * B_CORE + b:m * B_CORE + b + 1])
                t_tiles.append(t_m)

            e_rep = psum.tile([128, s_core], f32, tag="acc", name="e_rep")
            for m in range(4):
                for sh in range(n_sh):
                    nc.tensor.matmul(
                        e_rep[:, sh * 512:(sh + 1) * 512],
                        bwr_sb[:, m * 128:(m + 1) * 128],
                        t_tiles[m][:, sh * 512:(sh + 1) * 512],
                        start=(m == 0), stop=(m == 3))
            w_rep = wpool.tile([128, s_core], bf16, tag="w", name="w_rep")
            nc.scalar.activation(w_rep, e_rep, AF.Exp)

            for k in range(2):
                prod = ppool.tile([128, s_core], bf16, tag="prod",
                                  name="prod")
                nc.vector.affine_mul_reduce(
                    out=prod, accum_out=p_fin[k][:, b:b + 1],
                    in0=vv[o][k][:, h, :], in1=w_rep,
                    scale=1.0, bias=0.0)
            sescr = ppool.tile([128, s_core], bf16, tag="sescr",
                               name="sescr")
            nc.vector.tensor_scalar(
                sescr, w_rep, 1.0, None, op0=ALU.mult, op1=ALU.add,
                accum_out=se_fin[:, b:b + 1])

        for k in range(2):
            nc.sync.dma_start(p_d[k], p_fin[k])
        nc.sync.dma_start(se_d[:], se_fin[0:1, :])

    nc.compile()
    return nc


def _get_nc(s_core=S_CORE):
    if s_core not in _NC_CACHE:
        _NC_CACHE[s_core] = _build_nc(s_core)
    return _NC_CACHE[s_core]


def _host_smalls(h_t, W_h_w, W_h_b, W_w, W_b, beta_w):
    q = h_t[:, 0, :].astype(np.float64) @ W_h_w.T.astype(np.float64) \
        + W_h_b + W_b                                  # [b, beta]
    # per batch-half: qs[p, m*B_CORE+b] = q[bh*B_CORE+b, m*128+p]
    qs3 = q.T.reshape(4, 128, B).transpose(1, 0, 2)    # [128, 4, 64]
    qs_h = [np.ascontiguousarray(
        qs3[:, :, bh * B_CORE:(bh + 1) * B_CORE].reshape(128, 4 * B_CORE)
    ).astype(np.float32) for bh in range(N_BH)]
    wt = np.ascontiguousarray(
        W_w.T.reshape(2, 128, BETA).transpose(1, 0, 2).reshape(128, 2 * BETA)
    ).astype(BF16)
    bw = beta_w[0].astype(np.float32)
    bwr = np.ascontiguousarray(
        np.repeat(bw.reshape(4, 128).T[:, :, None], 128, axis=2).reshape(128, BETA)
    ).astype(BF16)
    return qs_h, wt, bwr


_PROFILE = False
_LAST_PERF = {}


def kernel(**inputs):
    from concourse.bass_utils import run_bass_kernel_spmd

    V = np.asarray(inputs["V"], dtype=np.float32)
    h_t = np.asarray(inputs["h_t"], dtype=np.float32)
    W_h_w = np.asarray(inputs["W_h_w"], dtype=np.float32)
    W_h_b = np.asarray(inputs["W_h_b"], dtype=np.float32)
    W_w = np.asarray(inputs["W_w"], dtype=np.float32)
    W_b = np.asarray(inputs["W_b"], dtype=np.float32)
    beta_w = np.asarray(inputs["beta_w"], dtype=np.float32)
    beta_b = np.asarray(inputs["beta_b"], dtype=np.float32)

    qs_h, wt, bwr = _host_smalls(h_t, W_h_w, W_h_b, W_w, W_b, beta_w)

    rows = HP // N_HPQ
    Vb = V.astype(BF16)
    in_maps = []
    core_meta = []
    for k in range(N_HPQ):
        Vq = Vb[k * rows:(k + 1) * rows].reshape(S_CORE, C_DIM, B)
        for bh in range(N_BH):
            # [s, c, b-half] -> [c, b, s] contiguous
            vk = np.ascontiguousarray(
                Vq[:, :, bh * B_CORE:(bh + 1) * B_CORE].transpose(1, 2, 0))
            in_maps.append({"v": vk, "wt": wt, "qs": qs_h[bh], "bwr": bwr})
            core_meta.append(bh)

    nc = _get_nc()
    res = run_bass_kernel_spmd(nc, in_maps, core_ids=list(range(NCORES)),
                               trace=_PROFILE)
    if _PROFILE:
        _LAST_PERF["exec_time_ns"] = res.exec_time_ns
        _LAST_PERF["trace"] = res.instructions_and_trace
    P = np.zeros((C_DIM, B), np.float64)
    SE = np.zeros((B,), np.float64)
    for bh, r in zip(core_meta, res.results):
        sl = slice(bh * B_CORE, (bh + 1) * B_CORE)
        P[:, sl] += r["p_out"].reshape(C_DIM, B_CORE)
        SE[sl] += r["se_out"][0]
    # softmax is shift-invariant so beta_b cancels; no max-sub needed (|E|<=~8)
    C = (P / SE).T.reshape(B, 1, C_DIM)
    return C.astype(np.float32)
